# revision 28
# baseline (speedup 1.0000x reference)
"""Trainium2 Bass kernel for nn_DTIModel (DTI pairwise message passing).

Sharding: data-parallel over batch B=8 across the 8 NeuronCores (one batch
element per core, weights replicated). The [Np*Nd, H] pairwise tensor is
never materialized in DRAM: it is produced, attended (D=3) and reduced
tile-by-tile entirely in SBUF.

Math notes (per batch element, H=128, Np=512, Nd=96, N=Np*Nd):
  prot = lrelu(PT @ p_W + p_b, .1) * pu[:,None]        [Np,H]
  drug = lrelu(DR @ d_W + d_b, .1) * du[:,None]        [Nd,H]
  pv = prot @ Wv_p ; dv = drug @ Wv_d + Wv_b           (Wv_b folded into dv)
  m0[i,j] = lrelu(pv[i] + dv[j], .1)
  loop d=0..2:  y = m @ Wvs_d + b_d ; a = y@att_d + att_b_d ; m += a*y
  global = 2*a2*y2 + m2 + a1*y1-chain (== ref's global_i + mij)
  g1 = sum_pairs relu(global @ W1a + b1a) @ W1b + N*W1b_b
  ... small MLP head ... -> scalar per batch element.

Implementation tricks:
  * alpha_d = att_d^T y_d + att_b_d == A_d^T m_d + c_d with
    A_d = Wvs_d @ att_d, c_d = b_d.att_d + att_b_d  (exact, linear algebra).
  * One matmul with stationary R_d (every column == A_d) yields
    B = broadcast(alpha row) over all 128 partitions directly in PSUM.
  * g_d = (B + c_d) * y_d in one fused scalar_tensor_tensor.
  * The d=2 "global_i + mij" doubling is folded into R_2 and c_2 (x2).
  * sum_pairs relu(...) @ W1b == (sum_pairs relu(...)) @ W1b + N*W1b_b:
    the W1b GEMM runs once on the [H] sum, not per pair.
  * relu+bias+pair-sum fused via tensor_scalar accum_out.
"""

import numpy as np

import concourse.bass as bass
import concourse.mybir as mybir
import concourse.tile as tile
from concourse.masks import make_identity

F32 = mybir.dt.float32
BF16 = mybir.dt.bfloat16
ALU = mybir.AluOpType
ACTF = mybir.ActivationFunctionType

H = 128
NP = 512
ND = 96
NPAIR = NP * ND  # 49152
N_CORES = 8

_INPUT_SPECS = {
    # per-core (batch-sharded) tensors
    "protein_features": ([NP, H], True),
    "drug_features": ([ND, H], True),
    "pu_mask": ([NP], True),
    "du_mask": ([ND], True),
    # replicated weights
    "d_W": ([H, H], False), "d_b": ([H], False),
    "p_W": ([H, H], False), "p_b": ([H], False),
    "Wv_p": ([H, H], False), "Wv_d": ([H, H], False), "Wv_b": ([H], False),
    "att_W": ([3, H, 1], False), "att_b": ([3, 1], False),
    "Wvs_W": ([3, H, H], False), "Wvs_b": ([3, H], False),
    "Wu_W": ([2 * H, H], False), "Wu_b": ([H], False),
    "W1a_W": ([H, H], False), "W1a_b": ([H], False),
    "W1b_W": ([H, H], False), "W1b_b": ([H], False),
    "W2a_W": ([H, 2 * H], False), "W2a_b": ([2 * H], False),
    "W2b_W": ([2 * H, H], False), "W2b_b": ([H], False),
    "W3_W": ([H, H], False), "W3_b": ([H], False),
    "Wu1a_W": ([2 * H, H], False), "Wu1a_b": ([H], False),
    "Wu1b_W": ([H, H], False), "Wu1b_b": ([H], False),
    "W5_W": ([H, 1], False), "W5_b": ([1], False),
}


_LCNT = [0]


def _lrelu_col(nc, pool, psum_col, bias_col, slope):
    """lrelu(psum_col + bias_col, slope) for a [128,1] column -> sbuf f32."""
    _LCNT[0] += 1
    x = pool.tile([H, 1], F32, tag="lx%d" % _LCNT[0])
    nc.vector.tensor_scalar(x, psum_col, bias_col, None, ALU.add)
    o = pool.tile([H, 1], F32, tag="lo%d" % _LCNT[0])
    nc.vector.scalar_tensor_tensor(o, x, slope, x, ALU.mult, ALU.max)
    return o


def _legalize_multiwaits(nc):
    """Walrus's codegen only supports ONE semaphore wait per instruction
    (NEURON_ISA_TPB_EVENTS has a single wait slot) and errors with "Too many
    sync wait commands" otherwise. Tile emits multi-wait sync_infos, so split
    them: hoist all but one wait onto single-wait EventSemaphore instructions
    immediately before the owner on the same engine queue."""
    uid = [0]
    for fn in nc.m.functions:
        for blk in fn.blocks:
            out = []
            changed = False
            for inst in blk.instructions:
                si = inst.sync_info
                waits = list(si.on_wait) if si is not None else []
                if len(waits) > 1:
                    for w in waits[:-1]:
                        uid[0] += 1
                        ev = mybir.InstEventSemaphore(
                            name="I-mwsplit-%d" % uid[0], ins=[], outs=[],
                            engine=inst.engine)
                        ev.sync_info = mybir.SyncInfo(on_wait=[w], on_update=[])
                        out.append(ev)
                    inst.sync_info = mybir.SyncInfo(
                        on_wait=[waits[-1]], on_update=list(si.on_update))
                    changed = True
                out.append(inst)
            if changed:
                blk.instructions = out


def build_bass():
    nc = bass.Bass()

    din = {}
    for name, (shape, _) in _INPUT_SPECS.items():
        din[name] = nc.declare_dram_parameter(name, list(shape), F32, isOutput=False)
    dout = nc.declare_dram_parameter("out", [1, 1], F32, isOutput=True)

    with tile.TileContext(nc) as tc:
        _emit(nc, tc, din, dout)
    _legalize_multiwaits(nc)
    return nc


def _emit(nc, tc, din, dout):
    import contextlib

    ctx = contextlib.ExitStack()
    with ctx:
        const = ctx.enter_context(tc.tile_pool(name="const", bufs=1))
        setup = ctx.enter_context(tc.tile_pool(name="setup", bufs=2))
        spsum_cm = tc.tile_pool(name="spsum", bufs=1, space="PSUM")
        spsum = spsum_cm.__enter__()

        # ---------------- load weights (f32, natural layout) -------------
        _uid = [0]

        def _tag(p):
            _uid[0] += 1
            return "%s%d" % (p, _uid[0])

        def load(name, src=None, shape=None):
            src = src if src is not None else din[name]
            if not isinstance(src, bass.AP):
                src = src[:]
            t = const.tile(shape or list(src.shape), F32, tag=_tag("w"))
            nc.sync.dma_start(out=t, in_=src)
            return t

        p_W = load("p_W"); d_W = load("d_W")
        Wv_p = load("Wv_p"); Wv_d = load("Wv_d")
        Wvs = [load(None, src=din["Wvs_W"][d]) for d in range(3)]
        attW = [load(None, src=din["att_W"][d]) for d in range(3)]  # [128,1]
        W1a = load("W1a_W")

        # bias vectors as [128,1] columns (partition-major)
        def load_col(src, n=H):
            if not isinstance(src, bass.AP):
                src = src[:]
            t = const.tile([n, 1], F32, tag=_tag("b"))
            nc.sync.dma_start(out=t, in_=src.rearrange("(n o) -> n o", o=1))
            return t

        p_b = load_col(din["p_b"]); d_b = load_col(din["d_b"])
        Wv_b = load_col(din["Wv_b"])
        Wvs_b = [load_col(din["Wvs_b"][d]) for d in range(3)]
        W1a_b = load_col(din["W1a_b"])
        att_b = [load_col(din["att_b"][d], n=1) for d in range(3)]  # [1,1]

        pu_row = const.tile([1, NP], F32)
        nc.sync.dma_start(
            out=pu_row, in_=din["pu_mask"][:].rearrange("(o n) -> o n", o=1))
        du_row = const.tile([1, ND], F32)
        nc.sync.dma_start(
            out=du_row, in_=din["du_mask"][:].rearrange("(o n) -> o n", o=1))

        ident = const.tile([H, H], F32)
        make_identity(nc, ident)
        ones_row = const.tile([1, H], F32)
        nc.vector.memset(ones_row, 1.0)
        ones_t16 = const.tile([H, H], BF16)
        nc.vector.memset(ones_t16, 1.0)

        # ---------------- transpose features: PTt [H, NP], DRt [H, ND] ----
        PTt = const.tile([H, NP], F32)
        for t in range(4):
            nat = setup.tile([H, H], F32, tag="nat")
            nc.sync.dma_start(out=nat, in_=din["protein_features"][t * H:(t + 1) * H, :])
            ps = spsum.tile([H, H], F32, tag="tps")
            nc.tensor.transpose(ps, nat, ident)
            nc.vector.tensor_copy(PTt[:, t * H:(t + 1) * H], ps)
        DRt = const.tile([H, ND], F32)
        natd = setup.tile([H, H], F32, tag="nat")
        nc.sync.dma_start(out=natd[0:ND, :], in_=din["drug_features"][:, :])
        psd = spsum.tile([H, H], F32, tag="tps")
        nc.tensor.transpose(psd[:, 0:ND], natd[0:ND, :], ident[0:ND, 0:ND])
        nc.vector.tensor_copy(DRt, psd[:, 0:ND])

        # ---------------- stage 1: protein / drug features ---------------
        # prot^T = lrelu(p_W^T @ PTt + p_b, .1) * pu
        def feat(WT, Xt, b_col, mask_row, n):
            ps = spsum.tile([H, NP], F32, tag="s1p")
            nc.tensor.matmul(ps[:, 0:n], WT, Xt, start=True, stop=True)
            x = setup.tile([H, NP], F32, tag="s1x")
            nc.vector.tensor_scalar(x[:, 0:n], ps[:, 0:n], b_col, None, ALU.add)
            l = setup.tile([H, NP], F32, tag="s1l")
            nc.vector.scalar_tensor_tensor(
                l[:, 0:n], x[:, 0:n], 0.1, x[:, 0:n], ALU.mult, ALU.max)
            pm = spsum.tile([H, NP], F32, tag="s1m")
            nc.tensor.matmul(pm[:, 0:n], ones_row, mask_row, start=True, stop=True)
            f = setup.tile([H, NP], F32, tag="s1f")
            nc.vector.scalar_tensor_tensor(
                f[:, 0:n], l[:, 0:n], 1.0, pm[:, 0:n], ALU.mult, ALU.mult)
            return f

        prot = feat(p_W, PTt, p_b, pu_row, NP)      # [128, 512] f32 (cols 0:512)
        drug = feat(d_W, DRt, d_b, du_row, ND)      # [128, 96]

        u_p = const.tile([H, 1], F32)
        nc.vector.tensor_reduce(u_p, prot[:, 0:NP], mybir.AxisListType.X, ALU.add)
        u_d = const.tile([H, 1], F32)
        nc.vector.tensor_reduce(u_d, drug[:, 0:ND], mybir.AxisListType.X, ALU.add)

        # pv = Wv_p^T @ prot (bias folded into dv); scaled bf16 copies
        ps_pv = spsum.tile([H, NP], F32, tag="s1p")
        nc.tensor.matmul(ps_pv, Wv_p, prot[:, 0:NP], start=True, stop=True)
        pv09 = const.tile([H, NP], BF16)
        nc.vector.tensor_scalar(pv09, ps_pv, 0.9, None, ALU.mult)
        pv01 = const.tile([H, NP], BF16)
        nc.scalar.activation(pv01, ps_pv, ACTF.Identity, scale=0.1)

        ps_dv = spsum.tile([H, ND], F32, tag="s1m")
        nc.tensor.matmul(ps_dv, Wv_d, drug[:, 0:ND], start=True, stop=True)
        dvf = setup.tile([H, ND], F32, tag="dvf")
        nc.vector.tensor_scalar(dvf, ps_dv, Wv_b, None, ALU.add)  # + Wv_b
        dv09 = const.tile([H, ND], F32)
        nc.vector.tensor_scalar(dv09, dvf, 0.9, None, ALU.mult)
        dv01 = const.tile([H, ND], F32)
        nc.vector.tensor_scalar(dv01, dvf, 0.1, None, ALU.mult)

        # ---------------- per-depth constants: A_d, R_d, c_d, biases -----
        Wvs16, R16, c_col, b_col = [], [], [], []
        for d in range(3):
            w16 = const.tile([H, H], BF16, tag=_tag("wvs16"))
            nc.vector.tensor_copy(w16, Wvs[d])
            Wvs16.append(w16)

            # A_d = Wvs_d @ att_d  (needs Wvs_d^T as stationary)
            psT = spsum.tile([H, H], F32, tag="tps")
            nc.tensor.transpose(psT, Wvs[d], ident)
            WvsT = setup.tile([H, H], F32, tag="wvsT")
            nc.vector.tensor_copy(WvsT, psT)
            psA = spsum.tile([H, 1], F32, tag="smu")
            nc.tensor.matmul(psA, WvsT, attW[d], start=True, stop=True)
            A_col = setup.tile([H, 1], F32, tag="acol")
            nc.vector.tensor_scalar(A_col, psA, 2.0 if d == 2 else 1.0, None, ALU.mult)
            R = const.tile([H, H], BF16, tag=_tag("R16"))
            nc.vector.tensor_scalar(R, ones_t16, A_col, None, ALU.mult)
            R16.append(R)

            # c_d = b_d . att_d + att_b_d   (x2 for d=2), replicated [128,1]
            psc = spsum.tile([1, 1], F32, tag="psc")
            nc.tensor.matmul(psc, Wvs_b[d], attW[d], start=True, stop=True)
            c1 = setup.tile([1, 1], F32, tag="c1")
            nc.vector.tensor_scalar(
                c1, psc, att_b[d], 2.0 if d == 2 else 1.0, ALU.add, ALU.mult)
            pscb = spsum.tile([H, 1], F32, tag="smu")
            nc.tensor.matmul(pscb, ones_row, c1, start=True, stop=True)
            cc = const.tile([H, 1], F32, tag=_tag("cc"))
            nc.vector.tensor_copy(cc, pscb)
            c_col.append(cc)
            b_col.append(Wvs_b[d])

        W1a16 = const.tile([H, H], BF16)
        nc.vector.tensor_copy(W1a16, W1a)

        sacc = const.tile([H, ND], F32)

        spsum_cm.__exit__(None, None, None)

        # ---------------- pairwise main loop (96 tiles of [128,512]) -----
        # m2/glob are never materialized: their consumers' matmuls accumulate
        # the m1/g1/g2 contributions directly in PSUM (PE has slack; the
        # elementwise engines are the bottleneck).
        # Two interleaved streams (even/odd j) with independent PSUM pools so
        # every engine always has a second tile's work ready while one tile's
        # serial chain stalls.
        with tc.tile_pool(name="pwork", bufs=3) as pw, \
             tc.tile_pool(name="psA", bufs=1, space="PSUM") as psA_y, \
             tc.tile_pool(name="psB", bufs=1, space="PSUM") as psA_b, \
             tc.tile_pool(name="psC", bufs=1, space="PSUM") as psB_y, \
             tc.tile_pool(name="psD", bufs=1, space="PSUM") as psB_b, \
             tc.tile_pool(name="psE", bufs=1, space="PSUM") as psC_y, \
             tc.tile_pool(name="psF", bufs=1, space="PSUM") as psC_b, \
             tc.tile_pool(name="psZ", bufs=2, space="PSUM") as ps_zp:
            pools = [(psA_y, psA_b), (psB_y, psB_b), (psC_y, psC_b)]

            def depth(st, ops, d):
                s = st["s"]; sp = st["sp"]
                pp_y, pp_b = pools[s]
                ps_y = pp_y.tile([H, NP], F32, tag="py" + sp)
                for i, t in enumerate(ops):
                    nc.tensor.matmul(ps_y, Wvs16[d], t,
                                     start=(i == 0), stop=(i == len(ops) - 1))
                ps_B = pp_b.tile([H, NP], F32, tag="pb" + sp)
                for i, t in enumerate(ops):
                    nc.tensor.matmul(ps_B, R16[d], t,
                                     start=(i == 0), stop=(i == len(ops) - 1))
                y = pw.tile([H, NP], BF16, tag="y%d%s" % (d, sp))
                if s == 0 and d == 0:
                    nc.vector.tensor_scalar(y, ps_y, b_col[d], None, ALU.add)
                else:
                    nc.scalar.activation(y, ps_y, ACTF.Identity, bias=b_col[d])
                g = pw.tile([H, NP], BF16, tag="g%d%s" % (d, sp))
                nc.vector.scalar_tensor_tensor(
                    g, ps_B, c_col[d], y, ALU.add, ALU.mult)
                return g

            def stage0(st):
                j, sp = st["j"], st["sp"]
                r = pw.tile([H, NP], BF16, tag="r" + sp)
                nc.vector.tensor_scalar(
                    r, pv09, dv09[:, j:j + 1], 0.0, ALU.add, ALU.max)
                m0 = pw.tile([H, NP], BF16, tag="m0" + sp)
                nc.vector.scalar_tensor_tensor(
                    m0, pv01, dv01[:, j:j + 1], r, ALU.add, ALU.add)
                st["m0"] = m0

            def stage1(st):
                st["g0"] = depth(st, [st["m0"]], 0)

            def stage2(st):
                sp = st["sp"]
                m1 = pw.tile([H, NP], BF16, tag="m1" + sp)
                nc.gpsimd.tensor_tensor(m1, st["m0"], st["g0"], ALU.add)
                st["m1"] = m1
                st["g1"] = depth(st, [m1], 1)

            def stage3(st):
                st["g2"] = depth(st, [st["m1"], st["g1"]], 2)

            def stage4(st):
                j, sp = st["j"], st["sp"]
                ps_z = ps_zp.tile([H, NP], F32, tag="pz")
                nc.tensor.matmul(ps_z, W1a16, st["g2"], start=True, stop=False)
                nc.tensor.matmul(ps_z, W1a16, st["g1"], start=False, stop=False)
                nc.tensor.matmul(ps_z, W1a16, st["m1"], start=False, stop=True)
                zs = pw.tile([H, NP], BF16, tag="zs" + sp)
                nc.scalar.activation(
                    zs, ps_z, ACTF.Relu, bias=W1a_b,
                    accum_out=sacc[:, j:j + 1])

            stages = [stage0, stage1, stage2, stage3, stage4]
            NSTG = len(stages)
            live = {}
            for jj in range(ND + 3 * (NSTG - 1)):
                # software pipeline: stage k of tile jj-2k this iteration,
                # deepest stage first
                for k in reversed(range(NSTG)):
                    j = jj - 3 * k
                    if 0 <= j < ND:
                        if k == 0:
                            live[j] = {"j": j, "s": j % 3, "sp": str(j % 3)}
                        stages[k](live[j])
                        if k == NSTG - 1:
                            del live[j]

        # ---------------- head (tiny, f32) --------------------------------
        # head-only weights are loaded here so their DMAs stay off the
        # critical setup path (HWDGE runs them during the pairwise loop).
        W1b = load("W1b_W"); W3 = load("W3_W")
        Wu1b = load("Wu1b_W"); W5 = load("W5_W")  # W5 [128,1]
        W2a_lo = load(None, src=din["W2a_W"][:, 0:H])
        W2a_hi = load(None, src=din["W2a_W"][:, H:2 * H])
        W2b_lo = load(None, src=din["W2b_W"][0:H, :])
        W2b_hi = load(None, src=din["W2b_W"][H:2 * H, :])
        Wu_lo = load(None, src=din["Wu_W"][0:H, :])
        Wu_hi = load(None, src=din["Wu_W"][H:2 * H, :])
        Wu1a_lo = load(None, src=din["Wu1a_W"][0:H, :])
        Wu1a_hi = load(None, src=din["Wu1a_W"][H:2 * H, :])
        Wu_b = load_col(din["Wu_b"]); W1b_b = load_col(din["W1b_b"])
        W2a_b_lo = load_col(din["W2a_b"][0:H]); W2a_b_hi = load_col(din["W2a_b"][H:2 * H])
        W2b_b = load_col(din["W2b_b"]); W3_b = load_col(din["W3_b"])
        Wu1a_b = load_col(din["Wu1a_b"]); Wu1b_b = load_col(din["Wu1b_b"])
        W5_b = load_col(din["W5_b"], n=1)          # [1,1]

        with tc.tile_pool(name="head", bufs=1) as hp, \
             tc.tile_pool(name="hpsum", bufs=1, space="PSUM") as hps:
            # m_u = lrelu(Wu^T [u_d;u_p] + Wu_b, .01)
            ps_mu = hps.tile([H, 1], F32, tag="h1")
            nc.tensor.matmul(ps_mu, Wu_lo, u_d, start=True, stop=False)
            nc.tensor.matmul(ps_mu, Wu_hi, u_p, start=False, stop=True)
            m_u = _lrelu_col(nc, hp, ps_mu, Wu_b, 0.01)

            s_col = hp.tile([H, 1], F32)
            nc.vector.tensor_reduce(s_col, sacc, mybir.AxisListType.X, ALU.add)

            ps_g1 = hps.tile([H, 1], F32, tag="h1")
            nc.tensor.matmul(ps_g1, W1b, s_col, start=True, stop=True)
            w1bbN = hp.tile([H, 1], F32)
            nc.vector.tensor_scalar(w1bbN, W1b_b, float(NPAIR), None, ALU.mult)
            g1 = hp.tile([H, 1], F32)
            nc.vector.tensor_scalar(g1, ps_g1, w1bbN, None, ALU.add)

            ps_lo = hps.tile([H, 1], F32, tag="h2")
            nc.tensor.matmul(ps_lo, W2a_lo, g1, start=True, stop=True)
            t_lo = _lrelu_col(nc, hp, ps_lo, W2a_b_lo, 0.1)
            ps_hi = hps.tile([H, 1], F32, tag="h3")
            nc.tensor.matmul(ps_hi, W2a_hi, g1, start=True, stop=True)
            t_hi = _lrelu_col(nc, hp, ps_hi, W2a_b_hi, 0.1)

            ps_g2 = hps.tile([H, 1], F32, tag="h4")
            nc.tensor.matmul(ps_g2, W2b_lo, t_lo, start=True, stop=False)
            nc.tensor.matmul(ps_g2, W2b_hi, t_hi, start=False, stop=True)
            g2 = hp.tile([H, 1], F32)
            nc.vector.tensor_scalar(g2, ps_g2, W2b_b, None, ALU.add)

            ps_g3 = hps.tile([H, 1], F32, tag="h5")
            nc.tensor.matmul(ps_g3, W3, g2, start=True, stop=True)
            g3 = _lrelu_col(nc, hp, ps_g3, W3_b, 0.1)

            ps_u = hps.tile([H, 1], F32, tag="h6")
            nc.tensor.matmul(ps_u, Wu1a_lo, m_u, start=True, stop=False)
            nc.tensor.matmul(ps_u, Wu1a_hi, g3, start=False, stop=True)
            h1 = _lrelu_col(nc, hp, ps_u, Wu1a_b, 0.1)

            ps_mu2 = hps.tile([H, 1], F32, tag="h7")
            nc.tensor.matmul(ps_mu2, Wu1b, h1, start=True, stop=True)
            mu = hp.tile([H, 1], F32)
            nc.vector.tensor_scalar(mu, ps_mu2, Wu1b_b, None, ALU.add)

            ps_o = hps.tile([1, 1], F32, tag="h8")
            nc.tensor.matmul(ps_o, W5, mu, start=True, stop=True)
            res = hp.tile([1, 1], F32)
            nc.vector.tensor_scalar(res, ps_o, W5_b, None, ALU.add)
            nc.sync.dma_start(out=dout[:, :], in_=res)


_CACHE = {}


def _get_nc():
    if "nc" not in _CACHE:
        _CACHE["nc"] = build_bass()
    return _CACHE["nc"]


def kernel(**inputs):
    from concourse.bass_utils import run_bass_kernel_spmd

    nc = _get_nc()
    per_core = {"protein_features", "drug_features", "pu_mask", "du_mask"}
    in_maps = []
    for b in range(N_CORES):
        m = {}
        for name in _INPUT_SPECS:
            arr = np.asarray(inputs[name], dtype=np.float32)
            m[name] = np.ascontiguousarray(arr[b]) if name in per_core else arr
        in_maps.append(m)
    res = run_bass_kernel_spmd(nc, in_maps, list(range(N_CORES)))
    out = np.stack([res.results[i]["out"].reshape(1) for i in range(N_CORES)])
    return out.astype(np.float32)


if __name__ == "__main__":
    nc = build_bass()
    print("build ok")


# revision 29
# speedup vs baseline: 2131.0051x; 2131.0051x over previous
"""Trainium2 Bass kernel for nn_DTIModel (DTI pairwise message passing).

Sharding: data-parallel over batch B=8 across the 8 NeuronCores (one batch
element per core, weights replicated). The [Np*Nd, H] pairwise tensor is
never materialized in DRAM: it is produced, attended (D=3) and reduced
tile-by-tile entirely in SBUF.

Math notes (per batch element, H=128, Np=512, Nd=96, N=Np*Nd):
  prot = lrelu(PT @ p_W + p_b, .1) * pu[:,None]        [Np,H]
  drug = lrelu(DR @ d_W + d_b, .1) * du[:,None]        [Nd,H]
  pv = prot @ Wv_p ; dv = drug @ Wv_d + Wv_b           (Wv_b folded into dv)
  m0[i,j] = lrelu(pv[i] + dv[j], .1)
  loop d=0..2:  y = m @ Wvs_d + b_d ; a = y@att_d + att_b_d ; m += a*y
  global = 2*a2*y2 + m2 + a1*y1-chain (== ref's global_i + mij)
  g1 = sum_pairs relu(global @ W1a + b1a) @ W1b + N*W1b_b
  ... small MLP head ... -> scalar per batch element.

Implementation tricks:
  * alpha_d = att_d^T y_d + att_b_d == A_d^T m_d + c_d with
    A_d = Wvs_d @ att_d, c_d = b_d.att_d + att_b_d  (exact, linear algebra).
  * One matmul with stationary R_d (every column == A_d) yields
    B = broadcast(alpha row) over all 128 partitions directly in PSUM.
  * g_d = (B + c_d) * y_d in one fused scalar_tensor_tensor.
  * The d=2 "global_i + mij" doubling is folded into R_2 and c_2 (x2).
  * sum_pairs relu(...) @ W1b == (sum_pairs relu(...)) @ W1b + N*W1b_b:
    the W1b GEMM runs once on the [H] sum, not per pair.
  * relu+bias+pair-sum fused via tensor_scalar accum_out.
"""

import numpy as np

import concourse.bass as bass
import concourse.mybir as mybir
import concourse.tile as tile
from concourse.masks import make_identity

F32 = mybir.dt.float32
BF16 = mybir.dt.bfloat16
ALU = mybir.AluOpType
ACTF = mybir.ActivationFunctionType

H = 128
NP = 512
ND = 96
NPAIR = NP * ND  # 49152
N_CORES = 8

_INPUT_SPECS = {
    # per-core (batch-sharded) tensors
    "protein_features": ([NP, H], True),
    "drug_features": ([ND, H], True),
    "pu_mask": ([NP], True),
    "du_mask": ([ND], True),
    # replicated weights
    "d_W": ([H, H], False), "d_b": ([H], False),
    "p_W": ([H, H], False), "p_b": ([H], False),
    "Wv_p": ([H, H], False), "Wv_d": ([H, H], False), "Wv_b": ([H], False),
    "att_W": ([3, H, 1], False), "att_b": ([3, 1], False),
    "Wvs_W": ([3, H, H], False), "Wvs_b": ([3, H], False),
    "Wu_W": ([2 * H, H], False), "Wu_b": ([H], False),
    "W1a_W": ([H, H], False), "W1a_b": ([H], False),
    "W1b_W": ([H, H], False), "W1b_b": ([H], False),
    "W2a_W": ([H, 2 * H], False), "W2a_b": ([2 * H], False),
    "W2b_W": ([2 * H, H], False), "W2b_b": ([H], False),
    "W3_W": ([H, H], False), "W3_b": ([H], False),
    "Wu1a_W": ([2 * H, H], False), "Wu1a_b": ([H], False),
    "Wu1b_W": ([H, H], False), "Wu1b_b": ([H], False),
    "W5_W": ([H, 1], False), "W5_b": ([1], False),
}


_LCNT = [0]


def _lrelu_col(nc, pool, psum_col, bias_col, slope):
    """lrelu(psum_col + bias_col, slope) for a [128,1] column -> sbuf f32."""
    _LCNT[0] += 1
    x = pool.tile([H, 1], F32, tag="lx%d" % _LCNT[0])
    nc.vector.tensor_scalar(x, psum_col, bias_col, None, ALU.add)
    o = pool.tile([H, 1], F32, tag="lo%d" % _LCNT[0])
    nc.vector.scalar_tensor_tensor(o, x, slope, x, ALU.mult, ALU.max)
    return o


def _legalize_multiwaits(nc):
    """Walrus's codegen only supports ONE semaphore wait per instruction
    (NEURON_ISA_TPB_EVENTS has a single wait slot) and errors with "Too many
    sync wait commands" otherwise. Tile emits multi-wait sync_infos, so split
    them: hoist all but one wait onto single-wait EventSemaphore instructions
    immediately before the owner on the same engine queue."""
    uid = [0]
    for fn in nc.m.functions:
        for blk in fn.blocks:
            out = []
            changed = False
            for inst in blk.instructions:
                si = inst.sync_info
                waits = list(si.on_wait) if si is not None else []
                if len(waits) > 1:
                    for w in waits[:-1]:
                        uid[0] += 1
                        ev = mybir.InstEventSemaphore(
                            name="I-mwsplit-%d" % uid[0], ins=[], outs=[],
                            engine=inst.engine)
                        ev.sync_info = mybir.SyncInfo(on_wait=[w], on_update=[])
                        out.append(ev)
                    inst.sync_info = mybir.SyncInfo(
                        on_wait=[waits[-1]], on_update=list(si.on_update))
                    changed = True
                out.append(inst)
            if changed:
                blk.instructions = out


def build_bass():
    nc = bass.Bass()

    din = {}
    for name, (shape, _) in _INPUT_SPECS.items():
        din[name] = nc.declare_dram_parameter(name, list(shape), F32, isOutput=False)
    dout = nc.declare_dram_parameter("out", [1, 1], F32, isOutput=True)

    with tile.TileContext(nc) as tc:
        _emit(nc, tc, din, dout)
    _legalize_multiwaits(nc)
    return nc


def _emit(nc, tc, din, dout):
    import contextlib

    ctx = contextlib.ExitStack()
    with ctx:
        const = ctx.enter_context(tc.tile_pool(name="const", bufs=1))
        setup = ctx.enter_context(tc.tile_pool(name="setup", bufs=2))
        spsum_cm = tc.tile_pool(name="spsum", bufs=1, space="PSUM")
        spsum = spsum_cm.__enter__()

        # ---------------- load weights (f32, natural layout) -------------
        _uid = [0]

        def _tag(p):
            _uid[0] += 1
            return "%s%d" % (p, _uid[0])

        def load(name, src=None, shape=None):
            src = src if src is not None else din[name]
            if not isinstance(src, bass.AP):
                src = src[:]
            t = const.tile(shape or list(src.shape), F32, tag=_tag("w"))
            nc.sync.dma_start(out=t, in_=src)
            return t

        p_W = load("p_W"); d_W = load("d_W")
        Wv_p = load("Wv_p"); Wv_d = load("Wv_d")
        Wvs = [load(None, src=din["Wvs_W"][d]) for d in range(3)]
        attW = [load(None, src=din["att_W"][d]) for d in range(3)]  # [128,1]
        W1a = load("W1a_W")

        # bias vectors as [128,1] columns (partition-major)
        def load_col(src, n=H):
            if not isinstance(src, bass.AP):
                src = src[:]
            t = const.tile([n, 1], F32, tag=_tag("b"))
            nc.sync.dma_start(out=t, in_=src.rearrange("(n o) -> n o", o=1))
            return t

        p_b = load_col(din["p_b"]); d_b = load_col(din["d_b"])
        Wv_b = load_col(din["Wv_b"])
        Wvs_b = [load_col(din["Wvs_b"][d]) for d in range(3)]
        W1a_b = load_col(din["W1a_b"])
        att_b = [load_col(din["att_b"][d], n=1) for d in range(3)]  # [1,1]

        pu_row = const.tile([1, NP], F32)
        nc.sync.dma_start(
            out=pu_row, in_=din["pu_mask"][:].rearrange("(o n) -> o n", o=1))
        du_row = const.tile([1, ND], F32)
        nc.sync.dma_start(
            out=du_row, in_=din["du_mask"][:].rearrange("(o n) -> o n", o=1))

        ident = const.tile([H, H], F32)
        make_identity(nc, ident)
        ones_row = const.tile([1, H], F32)
        nc.vector.memset(ones_row, 1.0)
        ones_t16 = const.tile([H, H], BF16)
        nc.vector.memset(ones_t16, 1.0)

        # ---------------- transpose features: PTt [H, NP], DRt [H, ND] ----
        PTt = const.tile([H, NP], F32)
        for t in range(4):
            nat = setup.tile([H, H], F32, tag="nat")
            nc.sync.dma_start(out=nat, in_=din["protein_features"][t * H:(t + 1) * H, :])
            ps = spsum.tile([H, H], F32, tag="tps")
            nc.tensor.transpose(ps, nat, ident)
            nc.vector.tensor_copy(PTt[:, t * H:(t + 1) * H], ps)
        DRt = const.tile([H, ND], F32)
        natd = setup.tile([H, H], F32, tag="nat")
        nc.sync.dma_start(out=natd[0:ND, :], in_=din["drug_features"][:, :])
        psd = spsum.tile([H, H], F32, tag="tps")
        nc.tensor.transpose(psd[:, 0:ND], natd[0:ND, :], ident[0:ND, 0:ND])
        nc.vector.tensor_copy(DRt, psd[:, 0:ND])

        # ---------------- stage 1: protein / drug features ---------------
        # prot^T = lrelu(p_W^T @ PTt + p_b, .1) * pu
        def feat(WT, Xt, b_col, mask_row, n):
            ps = spsum.tile([H, NP], F32, tag="s1p")
            nc.tensor.matmul(ps[:, 0:n], WT, Xt, start=True, stop=True)
            x = setup.tile([H, NP], F32, tag="s1x")
            nc.vector.tensor_scalar(x[:, 0:n], ps[:, 0:n], b_col, None, ALU.add)
            l = setup.tile([H, NP], F32, tag="s1l")
            nc.vector.scalar_tensor_tensor(
                l[:, 0:n], x[:, 0:n], 0.1, x[:, 0:n], ALU.mult, ALU.max)
            pm = spsum.tile([H, NP], F32, tag="s1m")
            nc.tensor.matmul(pm[:, 0:n], ones_row, mask_row, start=True, stop=True)
            f = setup.tile([H, NP], F32, tag="s1f")
            nc.vector.scalar_tensor_tensor(
                f[:, 0:n], l[:, 0:n], 1.0, pm[:, 0:n], ALU.mult, ALU.mult)
            return f

        prot = feat(p_W, PTt, p_b, pu_row, NP)      # [128, 512] f32 (cols 0:512)
        drug = feat(d_W, DRt, d_b, du_row, ND)      # [128, 96]

        u_p = const.tile([H, 1], F32)
        nc.vector.tensor_reduce(u_p, prot[:, 0:NP], mybir.AxisListType.X, ALU.add)
        u_d = const.tile([H, 1], F32)
        nc.vector.tensor_reduce(u_d, drug[:, 0:ND], mybir.AxisListType.X, ALU.add)

        # pv = Wv_p^T @ prot (bias folded into dv); scaled bf16 copies
        ps_pv = spsum.tile([H, NP], F32, tag="s1p")
        nc.tensor.matmul(ps_pv, Wv_p, prot[:, 0:NP], start=True, stop=True)
        pv09 = const.tile([H, NP], BF16)
        nc.vector.tensor_scalar(pv09, ps_pv, 0.9, None, ALU.mult)
        pv01 = const.tile([H, NP], BF16)
        nc.scalar.activation(pv01, ps_pv, ACTF.Identity, scale=0.1)

        ps_dv = spsum.tile([H, ND], F32, tag="s1m")
        nc.tensor.matmul(ps_dv, Wv_d, drug[:, 0:ND], start=True, stop=True)
        dvf = setup.tile([H, ND], F32, tag="dvf")
        nc.vector.tensor_scalar(dvf, ps_dv, Wv_b, None, ALU.add)  # + Wv_b
        dv09 = const.tile([H, ND], F32)
        nc.vector.tensor_scalar(dv09, dvf, 0.9, None, ALU.mult)
        dv01 = const.tile([H, ND], F32)
        nc.vector.tensor_scalar(dv01, dvf, 0.1, None, ALU.mult)

        # ---------------- per-depth constants: A_d, R_d, c_d, biases -----
        Wvs16, R16, c_col, b_col = [], [], [], []
        for d in range(3):
            w16 = const.tile([H, H], BF16, tag=_tag("wvs16"))
            nc.vector.tensor_copy(w16, Wvs[d])
            Wvs16.append(w16)

            # A_d = Wvs_d @ att_d  (needs Wvs_d^T as stationary)
            psT = spsum.tile([H, H], F32, tag="tps")
            nc.tensor.transpose(psT, Wvs[d], ident)
            WvsT = setup.tile([H, H], F32, tag="wvsT")
            nc.vector.tensor_copy(WvsT, psT)
            psA = spsum.tile([H, 1], F32, tag="smu")
            nc.tensor.matmul(psA, WvsT, attW[d], start=True, stop=True)
            A_col = setup.tile([H, 1], F32, tag="acol")
            nc.vector.tensor_scalar(A_col, psA, 2.0 if d == 2 else 1.0, None, ALU.mult)
            R = const.tile([H, H], BF16, tag=_tag("R16"))
            nc.vector.tensor_scalar(R, ones_t16, A_col, None, ALU.mult)
            R16.append(R)

            # c_d = b_d . att_d + att_b_d   (x2 for d=2), replicated [128,1]
            psc = spsum.tile([1, 1], F32, tag="psc")
            nc.tensor.matmul(psc, Wvs_b[d], attW[d], start=True, stop=True)
            c1 = setup.tile([1, 1], F32, tag="c1")
            nc.vector.tensor_scalar(
                c1, psc, att_b[d], 2.0 if d == 2 else 1.0, ALU.add, ALU.mult)
            pscb = spsum.tile([H, 1], F32, tag="smu")
            nc.tensor.matmul(pscb, ones_row, c1, start=True, stop=True)
            cc = const.tile([H, 1], F32, tag=_tag("cc"))
            nc.vector.tensor_copy(cc, pscb)
            c_col.append(cc)
            b_col.append(Wvs_b[d])

        W1a16 = const.tile([H, H], BF16)
        nc.vector.tensor_copy(W1a16, W1a)

        sacc = const.tile([H, ND], F32)

        spsum_cm.__exit__(None, None, None)

        # ---------------- pairwise main loop (96 tiles of [128,512]) -----
        # m2/glob are never materialized: their consumers' matmuls accumulate
        # the m1/g1/g2 contributions directly in PSUM (PE has slack; the
        # elementwise engines are the bottleneck).
        # Two interleaved streams (even/odd j) with independent PSUM pools so
        # every engine always has a second tile's work ready while one tile's
        # serial chain stalls.
        with tc.tile_pool(name="pwork", bufs=3) as pw, \
             tc.tile_pool(name="psA", bufs=1, space="PSUM") as psA_y, \
             tc.tile_pool(name="psB", bufs=1, space="PSUM") as psA_b, \
             tc.tile_pool(name="psC", bufs=1, space="PSUM") as psB_y, \
             tc.tile_pool(name="psD", bufs=1, space="PSUM") as psB_b, \
             tc.tile_pool(name="psE", bufs=1, space="PSUM") as psC_y, \
             tc.tile_pool(name="psF", bufs=1, space="PSUM") as psC_b, \
             tc.tile_pool(name="psZ", bufs=2, space="PSUM") as ps_zp:
            pools = [(psA_y, psA_b), (psB_y, psB_b), (psC_y, psC_b)]

            def depth(st, ops, d):
                s = st["s"]; sp = st["sp"]
                pp_y, pp_b = pools[s]
                ps_y = pp_y.tile([H, NP], F32, tag="py" + sp)
                for i, t in enumerate(ops):
                    nc.tensor.matmul(ps_y, Wvs16[d], t,
                                     start=(i == 0), stop=(i == len(ops) - 1))
                ps_B = pp_b.tile([H, NP], F32, tag="pb" + sp)
                for i, t in enumerate(ops):
                    nc.tensor.matmul(ps_B, R16[d], t,
                                     start=(i == 0), stop=(i == len(ops) - 1))
                y = pw.tile([H, NP], BF16, tag="y%d%s" % (d, sp))
                nc.scalar.activation(y, ps_y, ACTF.Identity, bias=b_col[d])
                g = pw.tile([H, NP], BF16, tag="g%d%s" % (d, sp))
                nc.vector.scalar_tensor_tensor(
                    g, ps_B, c_col[d], y, ALU.add, ALU.mult)
                return g

            def stage0(st):
                j, sp = st["j"], st["sp"]
                r = pw.tile([H, NP], BF16, tag="r" + sp)
                nc.vector.tensor_scalar(
                    r, pv09, dv09[:, j:j + 1], 0.0, ALU.add, ALU.max)
                m0 = pw.tile([H, NP], BF16, tag="m0" + sp)
                nc.vector.scalar_tensor_tensor(
                    m0, pv01, dv01[:, j:j + 1], r, ALU.add, ALU.add)
                st["m0"] = m0

            def stage1(st):
                st["g0"] = depth(st, [st["m0"]], 0)

            def stage2(st):
                sp = st["sp"]
                m1 = pw.tile([H, NP], BF16, tag="m1" + sp)
                nc.gpsimd.tensor_tensor(m1, st["m0"], st["g0"], ALU.add)
                st["m1"] = m1
                st["g1"] = depth(st, [m1], 1)

            def stage3(st):
                st["g2"] = depth(st, [st["m1"], st["g1"]], 2)

            def stage4(st):
                j, sp = st["j"], st["sp"]
                ps_z = ps_zp.tile([H, NP], F32, tag="pz")
                nc.tensor.matmul(ps_z, W1a16, st["g2"], start=True, stop=False)
                nc.tensor.matmul(ps_z, W1a16, st["g1"], start=False, stop=False)
                nc.tensor.matmul(ps_z, W1a16, st["m1"], start=False, stop=True)
                zs = pw.tile([H, NP], BF16, tag="zs" + sp)
                nc.scalar.activation(
                    zs, ps_z, ACTF.Relu, bias=W1a_b,
                    accum_out=sacc[:, j:j + 1])

            stages = [stage0, stage1, stage2, stage3, stage4]
            NSTG = len(stages)
            live = {}
            for jj in range(ND + 3 * (NSTG - 1)):
                # software pipeline: stage k of tile jj-2k this iteration,
                # deepest stage first
                for k in reversed(range(NSTG)):
                    j = jj - 3 * k
                    if 0 <= j < ND:
                        if k == 0:
                            live[j] = {"j": j, "s": j % 3, "sp": str(j % 3)}
                        stages[k](live[j])
                        if k == NSTG - 1:
                            del live[j]

        # ---------------- head (tiny, f32) --------------------------------
        # head-only weights are loaded here so their DMAs stay off the
        # critical setup path (HWDGE runs them during the pairwise loop).
        W1b = load("W1b_W"); W3 = load("W3_W")
        Wu1b = load("Wu1b_W"); W5 = load("W5_W")  # W5 [128,1]
        W2a_lo = load(None, src=din["W2a_W"][:, 0:H])
        W2a_hi = load(None, src=din["W2a_W"][:, H:2 * H])
        W2b_lo = load(None, src=din["W2b_W"][0:H, :])
        W2b_hi = load(None, src=din["W2b_W"][H:2 * H, :])
        Wu_lo = load(None, src=din["Wu_W"][0:H, :])
        Wu_hi = load(None, src=din["Wu_W"][H:2 * H, :])
        Wu1a_lo = load(None, src=din["Wu1a_W"][0:H, :])
        Wu1a_hi = load(None, src=din["Wu1a_W"][H:2 * H, :])
        Wu_b = load_col(din["Wu_b"]); W1b_b = load_col(din["W1b_b"])
        W2a_b_lo = load_col(din["W2a_b"][0:H]); W2a_b_hi = load_col(din["W2a_b"][H:2 * H])
        W2b_b = load_col(din["W2b_b"]); W3_b = load_col(din["W3_b"])
        Wu1a_b = load_col(din["Wu1a_b"]); Wu1b_b = load_col(din["Wu1b_b"])
        W5_b = load_col(din["W5_b"], n=1)          # [1,1]

        with tc.tile_pool(name="head", bufs=1) as hp, \
             tc.tile_pool(name="hpsum", bufs=1, space="PSUM") as hps:
            # m_u = lrelu(Wu^T [u_d;u_p] + Wu_b, .01)
            ps_mu = hps.tile([H, 1], F32, tag="h1")
            nc.tensor.matmul(ps_mu, Wu_lo, u_d, start=True, stop=False)
            nc.tensor.matmul(ps_mu, Wu_hi, u_p, start=False, stop=True)
            m_u = _lrelu_col(nc, hp, ps_mu, Wu_b, 0.01)

            s_col = hp.tile([H, 1], F32)
            nc.vector.tensor_reduce(s_col, sacc, mybir.AxisListType.X, ALU.add)

            ps_g1 = hps.tile([H, 1], F32, tag="h1")
            nc.tensor.matmul(ps_g1, W1b, s_col, start=True, stop=True)
            w1bbN = hp.tile([H, 1], F32)
            nc.vector.tensor_scalar(w1bbN, W1b_b, float(NPAIR), None, ALU.mult)
            g1 = hp.tile([H, 1], F32)
            nc.vector.tensor_scalar(g1, ps_g1, w1bbN, None, ALU.add)

            ps_lo = hps.tile([H, 1], F32, tag="h2")
            nc.tensor.matmul(ps_lo, W2a_lo, g1, start=True, stop=True)
            t_lo = _lrelu_col(nc, hp, ps_lo, W2a_b_lo, 0.1)
            ps_hi = hps.tile([H, 1], F32, tag="h3")
            nc.tensor.matmul(ps_hi, W2a_hi, g1, start=True, stop=True)
            t_hi = _lrelu_col(nc, hp, ps_hi, W2a_b_hi, 0.1)

            ps_g2 = hps.tile([H, 1], F32, tag="h4")
            nc.tensor.matmul(ps_g2, W2b_lo, t_lo, start=True, stop=False)
            nc.tensor.matmul(ps_g2, W2b_hi, t_hi, start=False, stop=True)
            g2 = hp.tile([H, 1], F32)
            nc.vector.tensor_scalar(g2, ps_g2, W2b_b, None, ALU.add)

            ps_g3 = hps.tile([H, 1], F32, tag="h5")
            nc.tensor.matmul(ps_g3, W3, g2, start=True, stop=True)
            g3 = _lrelu_col(nc, hp, ps_g3, W3_b, 0.1)

            ps_u = hps.tile([H, 1], F32, tag="h6")
            nc.tensor.matmul(ps_u, Wu1a_lo, m_u, start=True, stop=False)
            nc.tensor.matmul(ps_u, Wu1a_hi, g3, start=False, stop=True)
            h1 = _lrelu_col(nc, hp, ps_u, Wu1a_b, 0.1)

            ps_mu2 = hps.tile([H, 1], F32, tag="h7")
            nc.tensor.matmul(ps_mu2, Wu1b, h1, start=True, stop=True)
            mu = hp.tile([H, 1], F32)
            nc.vector.tensor_scalar(mu, ps_mu2, Wu1b_b, None, ALU.add)

            ps_o = hps.tile([1, 1], F32, tag="h8")
            nc.tensor.matmul(ps_o, W5, mu, start=True, stop=True)
            res = hp.tile([1, 1], F32)
            nc.vector.tensor_scalar(res, ps_o, W5_b, None, ALU.add)
            nc.sync.dma_start(out=dout[:, :], in_=res)


_CACHE = {}


def _get_nc():
    if "nc" not in _CACHE:
        _CACHE["nc"] = build_bass()
    return _CACHE["nc"]


def kernel(**inputs):
    from concourse.bass_utils import run_bass_kernel_spmd

    nc = _get_nc()
    per_core = {"protein_features", "drug_features", "pu_mask", "du_mask"}
    in_maps = []
    for b in range(N_CORES):
        m = {}
        for name in _INPUT_SPECS:
            arr = np.asarray(inputs[name], dtype=np.float32)
            m[name] = np.ascontiguousarray(arr[b]) if name in per_core else arr
        in_maps.append(m)
    res = run_bass_kernel_spmd(nc, in_maps, list(range(N_CORES)))
    out = np.stack([res.results[i]["out"].reshape(1) for i in range(N_CORES)])
    return out.astype(np.float32)


if __name__ == "__main__":
    nc = build_bass()
    print("build ok")
